# revision 8
# baseline (speedup 1.0000x reference)
"""BlockRecurrentTransformerLayer Trainium2 kernel.

Strategy (8 NeuronCores, no collectives — two SPMD launches + host gather):
  Phase 1 (token-parallel, 256 tokens/core): per-token projections
    xn=LN(x); ke/ve/qs/qx = xn@W; kn=LN(ke); rope; kh/kho/vh/qh/qxh head
    projections. Activation matmuls run weights-stationary producing
    feature-major (transposed) outputs that chain as the next matmul's
    moving operand.
  Host: gathers K-side tensors per batch, computes param-only constants
    (state-branch osh entirely, folded output projections, rope tables).
  Phase 2 (per core: one batch, query rows j::4 (strided — uniform causal
    work), recurrent-state chunk j): self-attention (causal + zero-cache
    sink), cross-attention, och attention, folded output projections, MLPs,
    gated state update. Attention computed in S^T orientation ([keys, q])
    so softmax denominators come from an appended ones-column through the
    AV matmul and probabilities never need transposing. Causal masks are
    data (0/1 tiles) so all 8 cores run one program.

All matmuls bf16 inputs / f32 PSUM accumulation.
"""
import numpy as np
import ml_dtypes

import concourse.bass as bass
import concourse.mybir as mybir
import concourse.tile as tile
from concourse import bacc
from concourse.bass_utils import run_bass_kernel_spmd
from concourse.masks import make_identity

F32 = mybir.dt.float32
BF16 = mybir.dt.bfloat16
BF = ml_dtypes.bfloat16
Exp = mybir.ActivationFunctionType.Exp
Relu = mybir.ActivationFunctionType.Relu
Sigmoid = mybir.ActivationFunctionType.Sigmoid
Tanh = mybir.ActivationFunctionType.Tanh
Identity = mybir.ActivationFunctionType.Identity
CopyF = mybir.ActivationFunctionType.Copy
Sqrt = mybir.ActivationFunctionType.Sqrt
ADD = mybir.AluOpType.add
MULT = mybir.AluOpType.mult
SUB = mybir.AluOpType.subtract

B, N, D, H, Dh, WIN, S = 2, 1024, 1024, 16, 64, 1024, 512
DM = H * Dh
NCORES = 8
TPC = 256          # tokens per core, phase 1
P = 128
EPS = 1e-5
ISCALE = 0.125     # 1/sqrt(Dh)

# self-attention per strided q-tile t: key blocks present / masked
SELF_NKC = [4, 8]               # q-tile 0 -> kc 0..3, q-tile 1 -> kc 0..7
SELF_MASKED = [[0, 1, 2, 3], [4, 5, 6, 7]]   # kc needing a 0/1 mask tile

_PROGS = {}


def _wst(w):
    """[K, M] -> [128, K/128, M] weights-stationary layout (lhsT blocks)."""
    k, m = w.shape
    return np.ascontiguousarray(
        w.reshape(k // P, P, m).transpose(1, 0, 2)).astype(BF)


def _bT(v):
    """[1024] bias -> [128, 8] per-partition chunks for fm evictions."""
    return np.ascontiguousarray(v.reshape(-1, P).T).astype(np.float32)


def _fm2full(a):
    """[128, C, T] feature-major chunks -> [C*128, T] full matrix."""
    p, c, t = a.shape
    return a.transpose(1, 0, 2).reshape(c * p, t)


def _full2fm(a):
    """[K, T] -> [128, K/128, T]."""
    k, t = a.shape
    return np.ascontiguousarray(a.reshape(k // P, P, t).transpose(1, 0, 2))


def _emit_ln(nc, pool, src, dst, eps_tile, nparts=P):
    """Gainless LayerNorm over the free dim of src [p, F] -> dst [p, F]."""
    Fd = src.shape[-1]
    nsub = Fd // 512
    stats = pool.tile([nparts, nsub, 6], F32, tag="ln_stats")
    for i in range(nsub):
        nc.vector.bn_stats(out=stats[:, i, :], in_=src[:, i * 512:(i + 1) * 512])
    mv = pool.tile([nparts, 2], F32, tag="ln_mv")
    nc.vector.bn_aggr(out=mv[:], in_=stats[:])
    rstd = pool.tile([nparts, 1], F32, tag="ln_rstd")
    nc.scalar.activation(out=rstd[:], in_=mv[:, 1:2], func=Sqrt,
                         bias=eps_tile[:nparts, :])
    nc.vector.reciprocal(rstd[:], rstd[:])
    nc.vector.tensor_scalar(out=dst, in0=src, scalar1=mv[:, 0:1],
                            scalar2=rstd[:], op0=SUB, op1=MULT)


# =====================================================================
# Phase-1 program
# =====================================================================
def _build_phase1():
    nc = bacc.Bacc("TRN2", target_bir_lowering=False, debug=False)

    def din(name, shape, dt):
        return nc.dram_tensor(name, shape, dt, kind="ExternalInput").ap()

    def dout(name, shape, dt):
        return nc.dram_tensor(name, shape, dt, kind="ExternalOutput").ap()

    x_d = din("x", [2, P, D], F32)
    tab_d = {nm: din(nm, [2, P, D], F32) for nm in ("cq", "sq", "ck", "sk")}
    gnT_d = din("gnT", [P, 8], F32)
    b_d = {nm: din("b_" + nm, [P, 8], F32)
           for nm in ("Ke", "Ve", "Qev", "Qsv", "k", "v", "q")}
    w_d = {nm: din("W_" + nm, [P, 8, DM], BF16)
           for nm in ("Ke", "Ve", "Qev", "Qsv", "k", "v", "q")}
    out_d = {nm: dout(nm, [P, 8, TPC], BF16)
             for nm in ("khT", "khoT", "vhT", "qhT", "qxhT")}
    xnT_d = dout("xnT", [P, 8, TPC], F32)

    with tile.TileContext(nc) as tc:
        with tc.tile_pool(name="const", bufs=1) as const, \
             tc.tile_pool(name="w", bufs=2) as wpool, \
             tc.tile_pool(name="act", bufs=1) as act, \
             tc.tile_pool(name="sm", bufs=2) as sm, \
             tc.tile_pool(name="ps", bufs=2, space="PSUM") as ps:

            ident_f = const.tile([P, P], F32)
            make_identity(nc, ident_f)
            ident_b = const.tile([P, P], BF16)
            make_identity(nc, ident_b)
            gnT = const.tile([P, 8], F32)
            nc.sync.dma_start(out=gnT[:], in_=gnT_d[:])
            eps_t = const.tile([P, 1], F32)
            nc.vector.memset(eps_t[:], EPS)
            bias_sb = {}
            for nm, d in b_d.items():
                t = const.tile([P, 8], F32, name=f"bias_{nm}")
                nc.sync.dma_start(out=t[:], in_=d[:])
                bias_sb[nm] = t
            tabs = {}
            for nm, d in tab_d.items():
                t = const.tile([P, 2, D], F32, name=f"tab_{nm}")
                nc.sync.dma_start(out=t[:], in_=d.rearrange("t p d -> p t d"))
                tabs[nm] = t

            def load_w(nm):
                t = wpool.tile([P, 8, DM], BF16, tag="w", name=f"w_{nm}")
                nc.sync.dma_start(out=t[:], in_=w_d[nm][:])
                return t

            def mm_ws(w_sb, rhs_fm):
                for mc in range(8):
                    pt = ps.tile([P, TPC], F32, tag="mm")
                    for kc in range(8):
                        nc.tensor.matmul(
                            pt[:], lhsT=w_sb[:, kc, mc * P:(mc + 1) * P],
                            rhs=rhs_fm[:, kc, :],
                            start=(kc == 0), stop=(kc == 7))
                    yield mc, pt

            def transpose_tm2fm(src_tm, dst_fm, ident):
                dt_ = src_tm.dtype
                for ti in range(2):
                    for dc in range(8):
                        pt = ps.tile([P, P], dt_, tag="tr")
                        nc.tensor.transpose(
                            pt[:], src_tm[:, ti, dc * P:(dc + 1) * P], ident[:])
                        nc.scalar.activation(
                            out=dst_fm[:, dc, ti * P:(ti + 1) * P],
                            in_=pt[:], func=CopyF)

            def transpose_fm2tm(src_fm, dst_tm):
                for ti in range(2):
                    for dc in range(8):
                        pt = ps.tile([P, P], F32, tag="tr")
                        nc.tensor.transpose(
                            pt[:], src_fm[:, dc, ti * P:(ti + 1) * P], ident_f[:])
                        nc.scalar.activation(
                            out=dst_tm[:, ti, dc * P:(dc + 1) * P],
                            in_=pt[:], func=CopyF)

            def rope(src_tm, ctab, stab, dst_tm_bf):
                for ti in range(2):
                    for half in range(2):
                        o = dst_tm_bf[:, ti, 512 * half:512 * half + 512]
                        a = src_tm[:, ti, 512 * half:512 * half + 512]
                        bsw = src_tm[:, ti, 512 * (1 - half):512 * (1 - half) + 512]
                        ch = ctab[:, ti, 512 * half:512 * half + 512]
                        sh = stab[:, ti, 512 * half:512 * half + 512]
                        t1 = sm.tile([P, 512], F32, tag="rope_t1")
                        nc.gpsimd.tensor_tensor(out=t1[:], in0=bsw, in1=sh, op=MULT)
                        t2 = sm.tile([P, 512], F32, tag="rope_t2")
                        nc.vector.tensor_tensor(out=t2[:], in0=a, in1=ch, op=MULT)
                        nc.vector.tensor_tensor(out=o, in0=t1[:], in1=t2[:], op=ADD)

            # ---- x -> LN -> gained transposes ----
            x_sb = act.tile([P, 2, D], F32, tag="tmA")
            for ti in range(2):
                nc.sync.dma_start(out=x_sb[:, ti, :], in_=x_d[ti])
            xn0 = act.tile([P, 2, D], F32, tag="tmB")
            for ti in range(2):
                _emit_ln(nc, sm, x_sb[:, ti, :], xn0[:, ti, :], eps_t)
            xnT_f = act.tile([P, 8, TPC], F32, tag="xnT_f")
            xnT_b = act.tile([P, 8, TPC], BF16, tag="xnT_b")
            for ti in range(2):
                for dc in range(8):
                    pt = ps.tile([P, P], F32, tag="tr")
                    nc.tensor.transpose(pt[:], xn0[:, ti, dc * P:(dc + 1) * P],
                                        ident_f[:])
                    nc.scalar.activation(out=xnT_b[:, dc, ti * P:(ti + 1) * P],
                                         in_=pt[:], func=CopyF,
                                         scale=gnT[:, dc:dc + 1])
                    nc.vector.tensor_scalar(
                        out=xnT_f[:, dc, ti * P:(ti + 1) * P], in0=pt[:],
                        scalar1=gnT[:, dc:dc + 1], scalar2=None, op0=MULT)
            nc.sync.dma_start(out=xnT_d[:], in_=xnT_f[:])

            # ---- the four xn projections ----
            w = load_w("Ke")
            ke_b = act.tile([P, 8, TPC], BF16, tag="ke_b")
            ke_f = act.tile([P, 8, TPC], F32, tag="ke_f")
            for mc, pt in mm_ws(w, xnT_b):
                nc.scalar.activation(out=ke_b[:, mc, :], in_=pt[:], func=Identity,
                                     bias=bias_sb["Ke"][:, mc:mc + 1])
                nc.vector.tensor_scalar(out=ke_f[:, mc, :], in0=pt[:],
                                        scalar1=bias_sb["Ke"][:, mc:mc + 1],
                                        scalar2=None, op0=ADD)
            w = load_w("Ve")
            ve_b = act.tile([P, 8, TPC], BF16, tag="ve_b")
            for mc, pt in mm_ws(w, xnT_b):
                nc.scalar.activation(out=ve_b[:, mc, :], in_=pt[:], func=Identity,
                                     bias=bias_sb["Ve"][:, mc:mc + 1])
            w = load_w("Qev")
            qs_f = act.tile([P, 8, TPC], F32, tag="qs_f")
            for mc, pt in mm_ws(w, xnT_b):
                nc.vector.tensor_scalar(out=qs_f[:, mc, :], in0=pt[:],
                                        scalar1=bias_sb["Qev"][:, mc:mc + 1],
                                        scalar2=None, op0=ADD)
            w = load_w("Qsv")
            qx_b = act.tile([P, 8, TPC], BF16, tag="qx_b")
            for mc, pt in mm_ws(w, xnT_b):
                nc.scalar.activation(out=qx_b[:, mc, :], in_=pt[:], func=Identity,
                                     bias=bias_sb["Qsv"][:, mc:mc + 1])

            # ---- ke -> LN -> rope -> fm ----
            ke_tm = act.tile([P, 2, D], F32, tag="tmA")
            transpose_fm2tm(ke_f, ke_tm)
            kn0 = act.tile([P, 2, D], F32, tag="tmB")
            for ti in range(2):
                _emit_ln(nc, sm, ke_tm[:, ti, :], kn0[:, ti, :], eps_t)
            kr = act.tile([P, 2, D], BF16, tag="ropeo")
            rope(kn0, tabs["ck"], tabs["sk"], kr)
            krT = act.tile([P, 8, TPC], BF16, tag="krT")
            transpose_tm2fm(kr, krT, ident_b)

            # ---- qs -> LN -> rope -> fm ----
            qs_tm = act.tile([P, 2, D], F32, tag="tmA")
            transpose_fm2tm(qs_f, qs_tm)
            qn0 = act.tile([P, 2, D], F32, tag="tmB")
            for ti in range(2):
                _emit_ln(nc, sm, qs_tm[:, ti, :], qn0[:, ti, :], eps_t)
            qr = act.tile([P, 2, D], BF16, tag="ropeo")
            rope(qn0, tabs["cq"], tabs["sq"], qr)
            qrT = act.tile([P, 8, TPC], BF16, tag="qrT")
            transpose_tm2fm(qr, qrT, ident_b)

            # ---- head projections (paired weight reuse) ----
            def dual_mm(wname, rhs1, rhs2, bias, out1_d, out2_d):
                w_ = load_w(wname)
                o1_ = act.tile([P, 8, TPC], BF16, tag=f"o_{wname}1")
                o2_ = act.tile([P, 8, TPC], BF16, tag=f"o_{wname}2")
                for mc in range(8):
                    pt1 = ps.tile([P, TPC], F32, tag="mm")
                    pt2 = ps.tile([P, TPC], F32, tag="mm2")
                    for kc in range(8):
                        lhsT = w_[:, kc, mc * P:(mc + 1) * P]
                        nc.tensor.matmul(pt1[:], lhsT=lhsT, rhs=rhs1[:, kc, :],
                                         start=(kc == 0), stop=(kc == 7))
                        nc.tensor.matmul(pt2[:], lhsT=lhsT, rhs=rhs2[:, kc, :],
                                         start=(kc == 0), stop=(kc == 7))
                    nc.scalar.activation(out=o1_[:, mc, :], in_=pt1[:],
                                         func=Identity, bias=bias[:, mc:mc + 1])
                    nc.scalar.activation(out=o2_[:, mc, :], in_=pt2[:],
                                         func=Identity, bias=bias[:, mc:mc + 1])
                nc.sync.dma_start(out=out1_d[:], in_=o1_[:])
                nc.sync.dma_start(out=out2_d[:], in_=o2_[:])

            dual_mm("k", krT, ke_b, bias_sb["k"], out_d["khT"], out_d["khoT"])
            dual_mm("q", qrT, qx_b, bias_sb["q"], out_d["qhT"], out_d["qxhT"])

            w = load_w("v")
            vhT = act.tile([P, 8, TPC], BF16, tag="vhT")
            for mc, pt in mm_ws(w, ve_b):
                nc.scalar.activation(out=vhT[:, mc, :], in_=pt[:], func=Identity,
                                     bias=bias_sb["v"][:, mc:mc + 1])
            nc.sync.dma_start(out=out_d["vhT"][:], in_=vhT[:])

    nc.compile()
    return nc


# =====================================================================
# Phase-2 program (uniform across cores; causal masks are data)
# =====================================================================
def _build_phase2():
    nc = bacc.Bacc("TRN2", target_bir_lowering=False, debug=False)

    def din(name, shape, dt):
        return nc.dram_tensor(name, shape, dt, kind="ExternalInput").ap()

    def dout(name, shape, dt):
        return nc.dram_tensor(name, shape, dt, kind="ExternalOutput").ap()

    qhT_d = din("qhT", [P, 8, 256], BF16)
    qxhT_d = din("qxhT", [P, 8, 256], BF16)
    xn_d = din("xn", [2, P, D], F32)
    p0b_d = din("p0b", [1, H, 256], BF16)
    p0f_d = din("p0f", [1, H, 256], F32)
    mask_d = din("mask", [P, 8, P], BF16)
    khT_d = din("khT", [P, 8, N], BF16)
    khoT_d = din("khoT", [P, 8, N], BF16)
    vhx_d = din("vhx", [P, 8, H * 65], BF16)
    vsink_d = din("vsink", [1, H * 65], BF16)
    kxhT_d = din("kxhT", [P, 8, S], BF16)
    vxx_d = din("vxx", [P, 4, H * 65], BF16)
    qchT_d = din("qchT", [P, 8, P], BF16)
    statec_d = din("statec", [P, D], F32)
    ohcT_d = din("ohcT", [P, 8, P], F32)
    gb1_d = din("gb1", [P, 3 * D], BF16)
    gb2_d = din("gb2", [P, 3 * D], BF16)
    bo1T_d = din("bo1T", [P, 8], F32)
    w_d = {}
    for nm, kc, m in (("Wfs", 8, D), ("Wfc", 8, D), ("Wmv1", 8, 2 * D),
                      ("Wmv2", 16, D), ("Wolh", 8, D), ("Wg1", 8, 3 * D),
                      ("Wg2", 8, 3 * D), ("Wmh1", 8, 2 * D), ("Wmh2", 16, D)):
        w_d[nm] = din(nm, [P, kc, m], BF16)
    o2_d = dout("o2", [2, P, D], F32)
    c2_d = dout("c2", [P, D], F32)

    with tile.TileContext(nc) as tc:
        with tc.tile_pool(name="sb", bufs=1) as sb, \
             tc.tile_pool(name="wt", bufs=2) as wt, \
             tc.tile_pool(name="sm", bufs=2) as sm, \
             tc.tile_pool(name="pp", bufs=2) as ppool, \
             tc.tile_pool(name="ps", bufs=2, space="PSUM") as ps:

            ident_f = sb.tile([P, P], F32, tag="identf")
            make_identity(nc, ident_f)
            ident_b = sb.tile([P, P], BF16, tag="identb")
            make_identity(nc, ident_b)
            eps_t = sb.tile([P, 1], F32, tag="eps")
            nc.vector.memset(eps_t[:], EPS)

            def ld(d_ap, shape, dt, tag, name=None):
                t = sb.tile(shape, dt, tag=tag, name=name or tag)
                nc.sync.dma_start(out=t[:], in_=d_ap[:])
                return t

            qhT = ld(qhT_d, [P, 8, 256], BF16, "qhT")
            qxhT = ld(qxhT_d, [P, 8, 256], BF16, "qxhT")
            qchT = ld(qchT_d, [P, 8, P], BF16, "qchT")
            p0b = ld(p0b_d, [1, H, 256], BF16, "p0b")
            p0f = ld(p0f_d, [1, H, 256], F32, "p0f")
            mask = ld(mask_d, [P, 8, P], BF16, "mask")
            vhx = ld(vhx_d, [P, 8, H * 65], BF16, "vhx")
            vsink = ld(vsink_d, [1, H * 65], BF16, "vsink")
            vxx = ld(vxx_d, [P, 4, H * 65], BF16, "vxx")

            def attention(out_sb, out_qsl, kT_sb, q_sb, qsl, v_sb, nkc, W_,
                          tag, masked=(), mask_of=None, sink=None):
                """S^T attention for all 16 heads -> out_sb[:, hp, out_qsl]."""
                for h in range(H):
                    hp, base = h // 2, 64 * (h % 2)
                    p_t = ppool.tile([P, nkc, W_], BF16, tag=f"p_{tag}")
                    for kc in range(nkc):
                        pt = ps.tile([P, 256], F32, tag="s")
                        nc.tensor.matmul(
                            pt[:, :W_],
                            lhsT=kT_sb[base:base + 64, hp, kc * P:(kc + 1) * P],
                            rhs=q_sb[base:base + 64, hp, qsl],
                            start=True, stop=True, tile_position=(base, 0))
                        nc.scalar.activation(out=p_t[:, kc, :], in_=pt[:, :W_],
                                             func=Exp, scale=ISCALE)
                        if kc in masked:
                            nc.vector.tensor_tensor(
                                out=p_t[:, kc, :], in0=p_t[:, kc, :],
                                in1=mask[:, mask_of[kc], :W_], op=MULT)
                    av = ps.tile([65, 256], F32, tag="av")
                    for kc in range(nkc):
                        nc.tensor.matmul(av[:, :W_],
                                         lhsT=v_sb[:, kc, 65 * h:65 * h + 65],
                                         rhs=p_t[:, kc, :],
                                         start=(kc == 0),
                                         stop=(kc == nkc - 1 and sink is None))
                    r = sm.tile([1, 256], F32, tag="r")
                    if sink is not None:
                        nc.tensor.matmul(av[:, :W_],
                                         lhsT=vsink[0:1, 65 * h:65 * h + 65],
                                         rhs=p0b[0:1, h, sink],
                                         start=False, stop=True)
                        nc.vector.tensor_tensor(out=r[:, :W_], in0=av[64:65, :W_],
                                                in1=p0f[0:1, h, sink], op=ADD)
                        nc.vector.reciprocal(r[:, :W_], r[:, :W_])
                    else:
                        nc.vector.reciprocal(r[:, :W_], av[64:65, :W_])
                    R = sm.tile([64, 256], F32, tag="R")
                    nc.gpsimd.partition_broadcast(R[:, :W_], r[:, :W_])
                    nc.vector.tensor_tensor(
                        out=out_sb[base:base + 64, hp, out_qsl],
                        in0=av[0:64, :W_], in1=R[:, :W_], op=MULT)

            # ---------------- self attention (strided causal) ----------------
            khT = ld(khT_d, [P, 8, N], BF16, "khT")
            attnS = sb.tile([P, 8, 256], BF16, tag="attnS")
            for t in range(2):
                qsl = slice(t * P, t * P + P)
                mask_of = {kc: SELF_MASKED[t].index(kc) + 4 * t
                           for kc in SELF_MASKED[t]}
                attention(attnS, qsl, khT, qhT, qsl, vhx, SELF_NKC[t], P,
                          "self", masked=SELF_MASKED[t], mask_of=mask_of,
                          sink=qsl)

            # ---------------- och (state queries x token keys) ----------------
            khoT = ld(khoT_d, [P, 8, N], BF16, "khoT")
            ochT = sb.tile([P, 8, P], BF16, tag="ochT")
            attention(ochT, slice(0, P), khoT, qchT, slice(0, P), vhx, 8, P,
                      "och")

            # ---------------- cross (token queries x state keys) --------------
            kxhT = ld(kxhT_d, [P, 8, S], BF16, "kxhT")
            attnC = sb.tile([P, 8, 256], BF16, tag="attnC")
            attention(attnC, slice(0, 256), kxhT, qxhT, slice(0, 256), vxx, 4,
                      256, "cross")

            # ---------------- o1 ----------------
            bo1T = ld(bo1T_d, [P, 8], F32, "bo1T")
            xn_sb = sb.tile([P, 2, D], F32, tag="xn")
            for ti in range(2):
                nc.sync.dma_start(out=xn_sb[:, ti, :], in_=xn_d[ti])

            def stream_w(nm, kcn, mc, width, bufs=3):
                t = wt.tile([P, kcn, width], BF16, tag="wmc", bufs=bufs,
                            name=f"{nm}_{mc}")
                nc.sync.dma_start(out=t[:],
                                  in_=w_d[nm][:, :, mc * width:(mc + 1) * width])
                return t

            o1T = sb.tile([P, 8, 256], F32, tag="o1T")
            for mc in range(8):
                wfs = stream_w("Wfs", 8, mc, P)
                wfc = stream_w("Wfc", 8, mc, P)
                pt = ps.tile([P, 256], F32, tag="mm")
                for kc in range(8):
                    nc.tensor.matmul(pt[:], lhsT=wfs[:, kc, :],
                                     rhs=attnS[:, kc, :], start=(kc == 0),
                                     stop=False)
                for kc in range(8):
                    nc.tensor.matmul(pt[:], lhsT=wfc[:, kc, :],
                                     rhs=attnC[:, kc, :], start=False,
                                     stop=(kc == 7))
                nc.scalar.activation(out=o1T[:, mc, :], in_=pt[:], func=Identity,
                                     bias=bo1T[:, mc:mc + 1])
            o1 = sb.tile([P, 2, D], F32, tag="o1")
            for ti in range(2):
                for dc in range(8):
                    pt = ps.tile([P, P], F32, tag="tr")
                    nc.tensor.transpose(pt[:], o1T[:, dc, ti * P:(ti + 1) * P],
                                        ident_f[:])
                    nc.vector.tensor_tensor(
                        out=o1[:, ti, dc * P:(dc + 1) * P], in0=pt[:],
                        in1=xn_sb[:, ti, dc * P:(dc + 1) * P], op=ADD)

            # ---------------- MLP_v ----------------
            ln1 = sb.tile([P, 2, D], BF16, tag="lnv")
            for ti in range(2):
                _emit_ln(nc, sm, o1[:, ti, :], ln1[:, ti, :], eps_t)
            ln1T = sb.tile([P, 8, 256], BF16, tag="attnS")   # reuse slot
            for ti in range(2):
                for dc in range(8):
                    pt = ps.tile([P, P], BF16, tag="tr")
                    nc.tensor.transpose(pt[:], ln1[:, ti, dc * P:(dc + 1) * P],
                                        ident_b[:])
                    nc.scalar.activation(out=ln1T[:, dc, ti * P:(ti + 1) * P],
                                         in_=pt[:], func=CopyF)
            h1T = sb.tile([P, 16, 256], BF16, tag="khoT")    # reuse slot
            for mc in range(16):
                wmv = stream_w("Wmv1", 8, mc, P)
                pt = ps.tile([P, 256], F32, tag="mm")
                for kc in range(8):
                    nc.tensor.matmul(pt[:], lhsT=wmv[:, kc, :],
                                     rhs=ln1T[:, kc, :], start=(kc == 0),
                                     stop=(kc == 7))
                nc.scalar.activation(out=h1T[:, mc, :], in_=pt[:], func=Relu)
            o2T = sb.tile([P, 8, 256], F32, tag="o1T")       # reuse slot
            for mc in range(8):
                wmv = wt.tile([P, 16, P], BF16, tag="wmc", bufs=3,
                              name=f"Wmv2_{mc}")
                nc.sync.dma_start(out=wmv[:],
                                  in_=w_d["Wmv2"][:, :, mc * P:(mc + 1) * P])
                pt = ps.tile([P, 256], F32, tag="mm")
                for kc in range(16):
                    nc.tensor.matmul(pt[:], lhsT=wmv[:, kc, :],
                                     rhs=h1T[:, kc, :], start=(kc == 0),
                                     stop=(kc == 15))
                nc.scalar.activation(out=o2T[:, mc, :], in_=pt[:], func=CopyF)
            o2 = sb.tile([P, 2, D], F32, tag="xn")           # reuse slot
            for ti in range(2):
                for dc in range(8):
                    pt = ps.tile([P, P], F32, tag="tr")
                    nc.tensor.transpose(pt[:], o2T[:, dc, ti * P:(ti + 1) * P],
                                        ident_f[:])
                    nc.vector.tensor_tensor(
                        out=o2[:, ti, dc * P:(dc + 1) * P], in0=pt[:],
                        in1=o1[:, ti, dc * P:(dc + 1) * P], op=ADD)
                nc.sync.dma_start(out=o2_d[ti], in_=o2[:, ti, :])

            # ---------------- oh ----------------
            ohcT = ld(ohcT_d, [P, 8, P], F32, "ohcT")
            ohT = sb.tile([P, 8, P], BF16, tag="ohT")
            for mc in range(8):
                wol = stream_w("Wolh", 8, mc, P)
                pt = ps.tile([P, 256], F32, tag="mm")
                for kc in range(8):
                    nc.tensor.matmul(pt[:, :P], lhsT=wol[:, kc, :],
                                     rhs=ochT[:, kc, :], start=(kc == 0),
                                     stop=(kc == 7))
                nc.vector.tensor_tensor(out=ohT[:, mc, :], in0=pt[:, :P],
                                        in1=ohcT[:, mc, :], op=ADD)

            # ---------------- gates + MLP_h ----------------
            statec = sb.tile([P, D], F32, tag="attnC")       # reuse slot
            nc.sync.dma_start(out=statec[:], in_=statec_d[:])

            def gate_block(src_fm, wname, gbd, cprev, out_c):
                gb = ld(gbd, [P, 3 * D], BF16, "gb", name=wname + "_gb")
                gsb = sb.tile([P, 3 * D], F32, tag="khT")    # reuse slot
                for nt in range(6):
                    wg = wt.tile([P, 8, 512], BF16, tag="wnt", bufs=2,
                                 name=f"{wname}_{nt}")
                    nc.sync.dma_start(
                        out=wg[:], in_=w_d[wname][:, :, nt * 512:(nt + 1) * 512])
                    pt = ps.tile([P, 512], F32, tag="mm")
                    for kc in range(8):
                        nc.tensor.matmul(pt[:], lhsT=src_fm[:, kc, :],
                                         rhs=wg[:, kc, :],
                                         start=(kc == 0), stop=(kc == 7))
                    pre = sm.tile([P, 512], F32, tag="gpre", bufs=2)
                    nc.vector.tensor_tensor(out=pre[:], in0=pt[:],
                                            in1=gb[:, nt * 512:(nt + 1) * 512],
                                            op=ADD)
                    func = Tanh if nt >= 4 else Sigmoid
                    nc.scalar.activation(out=gsb[:, nt * 512:(nt + 1) * 512],
                                         in_=pre[:], func=func)
                iz = sm.tile([P, D], F32, tag="iz", bufs=1)
                nc.vector.tensor_tensor(out=iz[:], in0=gsb[:, 0:D],
                                        in1=gsb[:, 2 * D:3 * D], op=MULT)
                nc.vector.tensor_tensor(out=out_c[:], in0=gsb[:, D:2 * D],
                                        in1=cprev[:], op=MULT)
                nc.vector.tensor_tensor(out=out_c[:], in0=out_c[:], in1=iz[:],
                                        op=ADD)

            c1 = sb.tile([P, D], F32, tag="qhT")             # reuse slot
            gate_block(ohT, "Wg1", gb1_d, statec, c1)

            lnh = sb.tile([P, D], BF16, tag="qchT")          # reuse slot
            _emit_ln(nc, sm, c1[:], lnh[:], eps_t)
            lnhT = sb.tile([P, 8, P], BF16, tag="mask")      # reuse slot
            for dc in range(8):
                pt = ps.tile([P, P], BF16, tag="tr")
                nc.tensor.transpose(pt[:], lnh[:, dc * P:(dc + 1) * P], ident_b[:])
                nc.scalar.activation(out=lnhT[:, dc, :], in_=pt[:], func=CopyF)
            hh = sb.tile([P, 2 * D], BF16, tag="kxhT")       # reuse slot
            for nt in range(4):
                wg = wt.tile([P, 8, 512], BF16, tag="wnt", bufs=2,
                             name=f"Wmh1_{nt}")
                nc.sync.dma_start(
                    out=wg[:], in_=w_d["Wmh1"][:, :, nt * 512:(nt + 1) * 512])
                pt = ps.tile([P, 512], F32, tag="mm")
                for kc in range(8):
                    nc.tensor.matmul(pt[:], lhsT=lnhT[:, kc, :],
                                     rhs=wg[:, kc, :],
                                     start=(kc == 0), stop=(kc == 7))
                nc.scalar.activation(out=hh[:, nt * 512:(nt + 1) * 512],
                                     in_=pt[:], func=Relu)
            hhT = sb.tile([P, 16, P], BF16, tag="vhx")       # reuse slot
            for dc in range(16):
                pt = ps.tile([P, P], BF16, tag="tr")
                nc.tensor.transpose(pt[:], hh[:, dc * P:(dc + 1) * P], ident_b[:])
                nc.scalar.activation(out=hhT[:, dc, :], in_=pt[:], func=CopyF)
            mT = sb.tile([P, 8, P], BF16, tag="ohT")         # reuse slot
            for mc in range(8):
                wm2 = wt.tile([P, 16, P], BF16, tag="wmc", bufs=3,
                              name=f"Wmh2_{mc}")
                nc.sync.dma_start(out=wm2[:],
                                  in_=w_d["Wmh2"][:, :, mc * P:(mc + 1) * P])
                pt = ps.tile([P, 256], F32, tag="mm")
                for kc in range(16):
                    nc.tensor.matmul(pt[:, :P], lhsT=wm2[:, kc, :],
                                     rhs=hhT[:, kc, :], start=(kc == 0),
                                     stop=(kc == 15))
                nc.scalar.activation(out=mT[:, mc, :], in_=pt[:, :P], func=CopyF)

            c2 = sb.tile([P, D], F32, tag="qxhT")            # reuse slot
            gate_block(mT, "Wg2", gb2_d, c1, c2)
            nc.sync.dma_start(out=c2_d[:], in_=c2[:])

    nc.compile()
    return nc


# =====================================================================
# Host orchestration
# =====================================================================
def _host_pre(x, freqs, Pm):
    c_t = np.cos(freqs[WIN:WIN + N])
    s_t = np.sin(freqs[WIN:WIN + N])
    sgn = np.concatenate([-np.ones(DM // 2, np.float32),
                          np.ones(DM // 2, np.float32)])
    s_signed = s_t * sgn

    def rope_tabs(g):
        gs = np.concatenate([g[DM // 2:], g[:DM // 2]])
        return c_t * g, s_signed * gs

    cq, sq = rope_tabs(Pm['g_qnorm'])
    ck, sk = rope_tabs(Pm['g_knorm'])

    def ln0(t):
        m = t.mean(-1, keepdims=True)
        v = ((t - m) ** 2).mean(-1, keepdims=True)
        return (t - m) / np.sqrt(v + EPS)

    def heads(t):
        return t.reshape(*t.shape[:-1], H, Dh)

    def softmax(sc):
        m = sc.max(-1, keepdims=True)
        e = np.exp(sc - m)
        return e / e.sum(-1, keepdims=True)

    state = Pm['state']
    spi = Pm['spi']
    scn = ln0(state) * Pm['g_state_norm']
    kx = (scn + spi) @ Pm['W_Ks'] + Pm['b_Ks']
    vx = (scn + spi) @ Pm['W_Vs'] + Pm['b_Vs']
    kxh = kx @ Pm['Wk'] + Pm['bk']
    vxh = vx @ Pm['Wv'] + Pm['bv']
    qch_h = ((state + spi) @ Pm['W_Qeh'] + Pm['b_Qeh']) @ Pm['Wq'] + Pm['bq']
    qsh_h = ((scn + spi) @ Pm['W_Qsh'] + Pm['b_Qsh']) @ Pm['Wq'] + Pm['bq']
    s_sh = np.einsum('shd,khd->hsk', heads(qsh_h), heads(kxh)) / np.sqrt(Dh)
    osh_attn = np.einsum('hsk,khd->shd', softmax(s_sh),
                         heads(vxh)).reshape(S, DM)
    b_oh = Pm['bo'] @ Pm['W_lh'][:DM] + Pm['bo'] @ Pm['W_lh'][DM:] + Pm['b_lh']
    oh_const = osh_attn @ (Pm['Wo'] @ Pm['W_lh'][:DM]) + b_oh

    return dict(
        cq=cq, sq=sq, ck=ck, sk=sk, kxh=kxh, vxh=vxh, qch_h=qch_h,
        oh_const=oh_const,
        Wo_lv_c=Pm['Wo'] @ Pm['W_lv'][:DM],
        Wo_lv_s=Pm['Wo'] @ Pm['W_lv'][DM:],
        b_o1=(Pm['bo'] @ Pm['W_lv'][:DM] + Pm['bo'] @ Pm['W_lv'][DM:]
              + Pm['b_lv']),
        Wo_lh_c=Pm['Wo'] @ Pm['W_lh'][DM:],
        W_mv1=Pm['g_mv'][:, None] * Pm['W_mv1'],
        W_mh1=Pm['g_mh'][:, None] * Pm['W_mh1'],
    )


def kernel(x, freqs, params):
    x = np.asarray(x, np.float32)
    freqs = np.asarray(freqs, np.float32)
    Pm = {k: np.asarray(v, np.float32) for k, v in params.items()}
    hp = _host_pre(x, freqs, Pm)

    # ---------------- phase 1 ----------------
    if "p1" not in _PROGS:
        _PROGS["p1"] = _build_phase1()
    w1 = {"W_Ke": _wst(Pm['W_Ke']), "W_Ve": _wst(Pm['W_Ve']),
          "W_Qev": _wst(Pm['W_Qev']), "W_Qsv": _wst(Pm['W_Qsv']),
          "W_k": _wst(Pm['Wk']), "W_v": _wst(Pm['Wv']), "W_q": _wst(Pm['Wq'])}
    b1 = {"b_Ke": _bT(Pm['b_Ke']), "b_Ve": _bT(Pm['b_Ve']),
          "b_Qev": _bT(Pm['b_Qev']), "b_Qsv": _bT(Pm['b_Qsv']),
          "b_k": _bT(Pm['bk']), "b_v": _bT(Pm['bv']), "b_q": _bT(Pm['bq']),
          "gnT": _bT(Pm['g_norm'])}
    in1 = []
    for core in range(NCORES):
        b = core // 4
        t0 = (core % 4) * TPC
        m = {"x": np.ascontiguousarray(x[b, t0:t0 + TPC].reshape(2, P, D))}
        for nm in ("cq", "sq", "ck", "sk"):
            m[nm] = np.ascontiguousarray(
                hp[nm][t0:t0 + TPC].reshape(2, P, D)).astype(np.float32)
        m.update(w1)
        m.update(b1)
        in1.append(m)
    r1 = run_bass_kernel_spmd(_PROGS["p1"], in1, core_ids=list(range(NCORES)))

    # ---------------- host mid ----------------
    def batch_cat(name, dtype):
        return [np.concatenate(
            [np.asarray(r1.results[4 * b + t][name], dtype) for t in range(4)],
            axis=2) for b in range(B)]

    khT_b = batch_cat("khT", np.float32)
    khoT_b = batch_cat("khoT", np.float32)
    qhT_b = batch_cat("qhT", np.float32)
    qxhT_b = batch_cat("qxhT", np.float32)
    vhT_b = batch_cat("vhT", np.float32)
    xnT_b = batch_cat("xnT", np.float32)

    def build_vext(v_tm, nkc):
        """v_tm [keys, DM] -> [128, nkc, H*65] with ones column."""
        ext = np.zeros((P, nkc, H * 65), np.float32)
        v3 = v_tm.reshape(nkc, P, H, Dh)
        for hh_ in range(H):
            ext[:, :, 65 * hh_:65 * hh_ + 64] = \
                v3[:, :, hh_, :].transpose(1, 0, 2)
            ext[:, :, 65 * hh_ + 64] = 1.0
        return ext

    vhx_np = [build_vext(_fm2full(vhT_b[b]).T, 8) for b in range(B)]
    vxx_np = build_vext(hp['vxh'], 4)
    vsink_np = np.zeros((1, H * 65), np.float32)
    for hh_ in range(H):
        vsink_np[0, 65 * hh_:65 * hh_ + 64] = Pm['bv'][64 * hh_:64 * hh_ + 64]

    p0_b = []
    for b in range(B):
        qh_tm = _fm2full(qhT_b[b]).T
        s0 = np.einsum('nhd,hd->nh', qh_tm.reshape(N, H, Dh),
                       Pm['bk'].reshape(H, Dh))
        p0 = (WIN - np.arange(N))[:, None] * np.exp(s0 * ISCALE)
        p0_b.append(p0.T)                                  # [H, N]

    # strided causal masks, identical structure for every core; values per j
    def build_mask(j):
        m = np.zeros((P, 8, P), np.float32)
        pp, ff = np.meshgrid(np.arange(P), np.arange(P), indexing='ij')
        idx = 0
        for t in range(2):
            for kc in SELF_MASKED[t]:
                m[:, idx, :] = ((kc * P + pp) <= (4 * (P * t + ff) + j))
                idx += 1
        return m

    gb1 = np.concatenate([Pm['b_i1'] - 1.0, Pm['b_f1'] + 1.0, Pm['b_z1']], 1)
    gb2 = np.concatenate([Pm['b_i2'] - 1.0, Pm['b_f2'] + 1.0, Pm['b_z2']], 1)
    w2 = {"Wfs": _wst(hp['Wo_lv_s']), "Wfc": _wst(hp['Wo_lv_c']),
          "Wmv1": _wst(hp['W_mv1']), "Wmv2": _wst(Pm['W_mv2']),
          "Wolh": _wst(hp['Wo_lh_c']),
          "Wg1": _wst(np.concatenate([Pm['W_i1'], Pm['W_f1'], Pm['W_z1']], 1)),
          "Wg2": _wst(np.concatenate([Pm['W_i2'], Pm['W_f2'], Pm['W_z2']], 1)),
          "Wmh1": _wst(hp['W_mh1']), "Wmh2": _wst(Pm['W_mh2'])}
    kxhT_fm = _full2fm(np.ascontiguousarray(hp['kxh'].T)).astype(BF)
    vxx_bf = vxx_np.astype(BF)

    if "p2" not in _PROGS:
        _PROGS["p2"] = _build_phase2()

    in2 = []
    for core in range(NCORES):
        b, jj = core // 4, core % 4
        cols = jj + 4 * np.arange(256)                     # strided q rows
        xn_tm = _fm2full(xnT_b[b]).T
        m = {
            "qhT": np.ascontiguousarray(qhT_b[b][:, :, cols]).astype(BF),
            "qxhT": np.ascontiguousarray(qxhT_b[b][:, :, cols]).astype(BF),
            "xn": np.ascontiguousarray(xn_tm[cols].reshape(2, P, D)),
            "p0b": np.ascontiguousarray(p0_b[b][None, :, cols]).astype(BF),
            "p0f": np.ascontiguousarray(p0_b[b][None, :, cols]).astype(np.float32),
            "mask": build_mask(jj).astype(BF),
            "khT": khT_b[b].astype(BF),
            "khoT": khoT_b[b].astype(BF),
            "vhx": vhx_np[b].astype(BF),
            "vsink": vsink_np.astype(BF),
            "kxhT": kxhT_fm,
            "vxx": vxx_bf,
            "qchT": _full2fm(np.ascontiguousarray(
                hp['qch_h'].T[:, jj * P:(jj + 1) * P])).astype(BF),
            "statec": np.ascontiguousarray(
                Pm['state'][jj * P:(jj + 1) * P]).astype(np.float32),
            "ohcT": _full2fm(np.ascontiguousarray(
                hp['oh_const'][jj * P:(jj + 1) * P].T)).astype(np.float32),
            "gb1": np.ascontiguousarray(gb1[jj * P:(jj + 1) * P]).astype(BF),
            "gb2": np.ascontiguousarray(gb2[jj * P:(jj + 1) * P]).astype(BF),
            "bo1T": _bT(hp['b_o1']),
        }
        m.update(w2)
        in2.append(m)

    r2 = run_bass_kernel_spmd(_PROGS["p2"], in2, core_ids=list(range(NCORES)))

    out = np.zeros((B, N + S, D), np.float32)
    for core in range(NCORES):
        b, jj = core // 4, core % 4
        rows = jj + 4 * np.arange(256)
        o2 = np.asarray(r2.results[core]["o2"], np.float32).reshape(256, D)
        out[b, rows] = o2
        out[b, N + jj * P:N + (jj + 1) * P] = \
            np.asarray(r2.results[core]["c2"], np.float32)
    return out


# revision 9
# speedup vs baseline: 23744.6653x; 23744.6653x over previous
"""BlockRecurrentTransformerLayer Trainium2 kernel.

Strategy (8 NeuronCores, no collectives — two SPMD launches + host gather):
  Phase 1 (token-parallel, 256 tokens/core): per-token projections
    xn=LN(x); ke/ve/qs/qx = xn@W; kn=LN(ke); rope; kh/kho/vh/qh/qxh head
    projections. Activation matmuls run weights-stationary producing
    feature-major (transposed) outputs that chain as the next matmul's
    moving operand.
  Host: gathers K-side tensors per batch, computes param-only constants
    (state-branch osh entirely, folded output projections, rope tables).
  Phase 2 (per core: one batch, query rows j::4 (strided — uniform causal
    work), recurrent-state chunk j): self-attention (causal + zero-cache
    sink), cross-attention, och attention, folded output projections, MLPs,
    gated state update. Attention computed in S^T orientation ([keys, q])
    so softmax denominators come from an appended ones-column through the
    AV matmul and probabilities never need transposing. Causal masks are
    data (0/1 tiles) so all 8 cores run one program.

All matmuls bf16 inputs / f32 PSUM accumulation.
"""
import numpy as np
import ml_dtypes

import concourse.bass as bass
import concourse.mybir as mybir
import concourse.tile as tile
from concourse import bacc
from concourse.bass_utils import run_bass_kernel_spmd
from concourse.masks import make_identity

F32 = mybir.dt.float32
BF16 = mybir.dt.bfloat16
BF = ml_dtypes.bfloat16
Exp = mybir.ActivationFunctionType.Exp
Relu = mybir.ActivationFunctionType.Relu
Sigmoid = mybir.ActivationFunctionType.Sigmoid
Tanh = mybir.ActivationFunctionType.Tanh
Identity = mybir.ActivationFunctionType.Identity
CopyF = mybir.ActivationFunctionType.Copy
Sqrt = mybir.ActivationFunctionType.Sqrt
ADD = mybir.AluOpType.add
MULT = mybir.AluOpType.mult
SUB = mybir.AluOpType.subtract

B, N, D, H, Dh, WIN, S = 2, 1024, 1024, 16, 64, 1024, 512
DM = H * Dh
NCORES = 8
TPC = 256          # tokens per core, phase 1
P = 128
EPS = 1e-5
ISCALE = 0.125     # 1/sqrt(Dh)

# self-attention per strided q-tile t: key blocks present / masked
SELF_NKC = [4, 8]               # q-tile 0 -> kc 0..3, q-tile 1 -> kc 0..7
SELF_MASKED = [[0, 1, 2, 3], [4, 5, 6, 7]]   # kc needing a 0/1 mask tile

_PROGS = {}

import contextlib as _cl


def _nullctx():
    return _cl.nullcontext()


def _wst(w):
    """[K, M] -> [128, K/128, M] weights-stationary layout (lhsT blocks)."""
    k, m = w.shape
    return np.ascontiguousarray(
        w.reshape(k // P, P, m).transpose(1, 0, 2)).astype(BF)


def _bT(v):
    """[1024] bias -> [128, 8] per-partition chunks for fm evictions."""
    return np.ascontiguousarray(v.reshape(-1, P).T).astype(np.float32)


def _fm2full(a):
    """[128, C, T] feature-major chunks -> [C*128, T] full matrix."""
    p, c, t = a.shape
    return a.transpose(1, 0, 2).reshape(c * p, t)


def _full2fm(a):
    """[K, T] -> [128, K/128, T]."""
    k, t = a.shape
    return np.ascontiguousarray(a.reshape(k // P, P, t).transpose(1, 0, 2))


def _emit_ln(nc, pool, src, dst, eps_tile, nparts=P):
    """Gainless LayerNorm over the free dim of src [p, F] -> dst [p, F]."""
    Fd = src.shape[-1]
    nsub = Fd // 512
    stats = pool.tile([nparts, nsub, 6], F32, tag="ln_stats")
    for i in range(nsub):
        nc.vector.bn_stats(out=stats[:, i, :], in_=src[:, i * 512:(i + 1) * 512])
    mv = pool.tile([nparts, 2], F32, tag="ln_mv")
    nc.vector.bn_aggr(out=mv[:], in_=stats[:])
    rstd = pool.tile([nparts, 1], F32, tag="ln_rstd")
    nc.scalar.activation(out=rstd[:], in_=mv[:, 1:2], func=Sqrt,
                         bias=eps_tile[:nparts, :])
    nc.vector.reciprocal(rstd[:], rstd[:])
    nc.vector.tensor_scalar(out=dst, in0=src, scalar1=mv[:, 0:1],
                            scalar2=rstd[:], op0=SUB, op1=MULT)


# =====================================================================
# Phase-1 program
# =====================================================================
def _build_phase1(repeat=1):
    nc = bacc.Bacc("TRN2", target_bir_lowering=False, debug=False)

    def din(name, shape, dt):
        return nc.dram_tensor(name, shape, dt, kind="ExternalInput").ap()

    def dout(name, shape, dt):
        return nc.dram_tensor(name, shape, dt, kind="ExternalOutput").ap()

    x_d = din("x", [2, P, D], F32)
    tab_d = {nm: din(nm, [2, P, D], F32) for nm in ("cq", "sq", "ck", "sk")}
    gnT_d = din("gnT", [P, 8], F32)
    b_d = {nm: din("b_" + nm, [P, 8], F32)
           for nm in ("Ke", "Ve", "Qev", "Qsv", "k", "v", "q")}
    w_d = {nm: din("W_" + nm, [P, 8, DM], BF16)
           for nm in ("Ke", "Ve", "Qev", "Qsv", "k", "v", "q")}
    out_d = {nm: dout(nm, [P, 8, TPC], BF16)
             for nm in ("khT", "khoT", "vhT", "qhT", "qxhT")}
    xnT_d = dout("xnT", [P, 8, TPC], F32)

    with tile.TileContext(nc) as tc:
        with tc.tile_pool(name="const", bufs=1) as const, \
             tc.tile_pool(name="w", bufs=2) as wpool, \
             tc.tile_pool(name="act", bufs=1) as act, \
             tc.tile_pool(name="sm", bufs=2) as sm, \
             tc.tile_pool(name="ps", bufs=2, space="PSUM") as ps, \
             (tc.For_i(0, repeat, 1) if repeat > 1 else _nullctx()):

            ident_f = const.tile([P, P], F32)
            make_identity(nc, ident_f)
            ident_b = const.tile([P, P], BF16)
            make_identity(nc, ident_b)
            gnT = const.tile([P, 8], F32)
            nc.sync.dma_start(out=gnT[:], in_=gnT_d[:])
            eps_t = const.tile([P, 1], F32)
            nc.vector.memset(eps_t[:], EPS)
            bias_sb = {}
            for nm, d in b_d.items():
                t = const.tile([P, 8], F32, name=f"bias_{nm}")
                nc.sync.dma_start(out=t[:], in_=d[:])
                bias_sb[nm] = t
            tabs = {}
            for nm, d in tab_d.items():
                t = const.tile([P, 2, D], F32, name=f"tab_{nm}")
                nc.sync.dma_start(out=t[:], in_=d.rearrange("t p d -> p t d"))
                tabs[nm] = t

            def load_w(nm):
                t = wpool.tile([P, 8, DM], BF16, tag="w", name=f"w_{nm}")
                nc.sync.dma_start(out=t[:], in_=w_d[nm][:])
                return t

            def mm_ws(w_sb, rhs_fm):
                for mc in range(8):
                    pt = ps.tile([P, TPC], F32, tag="mm")
                    for kc in range(8):
                        nc.tensor.matmul(
                            pt[:], lhsT=w_sb[:, kc, mc * P:(mc + 1) * P],
                            rhs=rhs_fm[:, kc, :],
                            start=(kc == 0), stop=(kc == 7))
                    yield mc, pt

            def transpose_tm2fm(src_tm, dst_fm, ident):
                dt_ = src_tm.dtype
                for ti in range(2):
                    for dc in range(8):
                        pt = ps.tile([P, P], dt_, tag="tr")
                        nc.tensor.transpose(
                            pt[:], src_tm[:, ti, dc * P:(dc + 1) * P], ident[:])
                        nc.scalar.activation(
                            out=dst_fm[:, dc, ti * P:(ti + 1) * P],
                            in_=pt[:], func=CopyF)

            def transpose_fm2tm(src_fm, dst_tm):
                for ti in range(2):
                    for dc in range(8):
                        pt = ps.tile([P, P], F32, tag="tr")
                        nc.tensor.transpose(
                            pt[:], src_fm[:, dc, ti * P:(ti + 1) * P], ident_f[:])
                        nc.scalar.activation(
                            out=dst_tm[:, ti, dc * P:(dc + 1) * P],
                            in_=pt[:], func=CopyF)

            def rope(src_tm, ctab, stab, dst_tm_bf):
                for ti in range(2):
                    for half in range(2):
                        o = dst_tm_bf[:, ti, 512 * half:512 * half + 512]
                        a = src_tm[:, ti, 512 * half:512 * half + 512]
                        bsw = src_tm[:, ti, 512 * (1 - half):512 * (1 - half) + 512]
                        ch = ctab[:, ti, 512 * half:512 * half + 512]
                        sh = stab[:, ti, 512 * half:512 * half + 512]
                        t1 = sm.tile([P, 512], F32, tag="rope_t1")
                        nc.gpsimd.tensor_tensor(out=t1[:], in0=bsw, in1=sh, op=MULT)
                        t2 = sm.tile([P, 512], F32, tag="rope_t2")
                        nc.vector.tensor_tensor(out=t2[:], in0=a, in1=ch, op=MULT)
                        nc.vector.tensor_tensor(out=o, in0=t1[:], in1=t2[:], op=ADD)

            # ---- x -> LN -> gained transposes ----
            x_sb = act.tile([P, 2, D], F32, tag="tmA")
            for ti in range(2):
                nc.sync.dma_start(out=x_sb[:, ti, :], in_=x_d[ti])
            xn0 = act.tile([P, 2, D], F32, tag="tmB")
            for ti in range(2):
                _emit_ln(nc, sm, x_sb[:, ti, :], xn0[:, ti, :], eps_t)
            xnT_f = act.tile([P, 8, TPC], F32, tag="xnT_f")
            xnT_b = act.tile([P, 8, TPC], BF16, tag="xnT_b")
            for ti in range(2):
                for dc in range(8):
                    pt = ps.tile([P, P], F32, tag="tr")
                    nc.tensor.transpose(pt[:], xn0[:, ti, dc * P:(dc + 1) * P],
                                        ident_f[:])
                    nc.scalar.activation(out=xnT_b[:, dc, ti * P:(ti + 1) * P],
                                         in_=pt[:], func=CopyF,
                                         scale=gnT[:, dc:dc + 1])
                    nc.vector.tensor_scalar(
                        out=xnT_f[:, dc, ti * P:(ti + 1) * P], in0=pt[:],
                        scalar1=gnT[:, dc:dc + 1], scalar2=None, op0=MULT)
            nc.sync.dma_start(out=xnT_d[:], in_=xnT_f[:])

            # ---- the four xn projections ----
            w = load_w("Ke")
            ke_b = act.tile([P, 8, TPC], BF16, tag="ke_b")
            ke_f = act.tile([P, 8, TPC], F32, tag="ke_f")
            for mc, pt in mm_ws(w, xnT_b):
                nc.scalar.activation(out=ke_b[:, mc, :], in_=pt[:], func=Identity,
                                     bias=bias_sb["Ke"][:, mc:mc + 1])
                nc.vector.tensor_scalar(out=ke_f[:, mc, :], in0=pt[:],
                                        scalar1=bias_sb["Ke"][:, mc:mc + 1],
                                        scalar2=None, op0=ADD)
            w = load_w("Ve")
            ve_b = act.tile([P, 8, TPC], BF16, tag="ve_b")
            for mc, pt in mm_ws(w, xnT_b):
                nc.scalar.activation(out=ve_b[:, mc, :], in_=pt[:], func=Identity,
                                     bias=bias_sb["Ve"][:, mc:mc + 1])
            w = load_w("Qev")
            qs_f = act.tile([P, 8, TPC], F32, tag="qs_f")
            for mc, pt in mm_ws(w, xnT_b):
                nc.vector.tensor_scalar(out=qs_f[:, mc, :], in0=pt[:],
                                        scalar1=bias_sb["Qev"][:, mc:mc + 1],
                                        scalar2=None, op0=ADD)
            w = load_w("Qsv")
            qx_b = act.tile([P, 8, TPC], BF16, tag="qx_b")
            for mc, pt in mm_ws(w, xnT_b):
                nc.scalar.activation(out=qx_b[:, mc, :], in_=pt[:], func=Identity,
                                     bias=bias_sb["Qsv"][:, mc:mc + 1])

            # ---- ke -> LN -> rope -> fm ----
            ke_tm = act.tile([P, 2, D], F32, tag="tmA")
            transpose_fm2tm(ke_f, ke_tm)
            kn0 = act.tile([P, 2, D], F32, tag="tmB")
            for ti in range(2):
                _emit_ln(nc, sm, ke_tm[:, ti, :], kn0[:, ti, :], eps_t)
            kr = act.tile([P, 2, D], BF16, tag="ropeo")
            rope(kn0, tabs["ck"], tabs["sk"], kr)
            krT = act.tile([P, 8, TPC], BF16, tag="krT")
            transpose_tm2fm(kr, krT, ident_b)

            # ---- qs -> LN -> rope -> fm ----
            qs_tm = act.tile([P, 2, D], F32, tag="tmA")
            transpose_fm2tm(qs_f, qs_tm)
            qn0 = act.tile([P, 2, D], F32, tag="tmB")
            for ti in range(2):
                _emit_ln(nc, sm, qs_tm[:, ti, :], qn0[:, ti, :], eps_t)
            qr = act.tile([P, 2, D], BF16, tag="ropeo")
            rope(qn0, tabs["cq"], tabs["sq"], qr)
            qrT = act.tile([P, 8, TPC], BF16, tag="qrT")
            transpose_tm2fm(qr, qrT, ident_b)

            # ---- head projections (paired weight reuse) ----
            def dual_mm(wname, rhs1, rhs2, bias, out1_d, out2_d):
                w_ = load_w(wname)
                o1_ = act.tile([P, 8, TPC], BF16, tag=f"o_{wname}1")
                o2_ = act.tile([P, 8, TPC], BF16, tag=f"o_{wname}2")
                for mc in range(8):
                    pt1 = ps.tile([P, TPC], F32, tag="mm")
                    pt2 = ps.tile([P, TPC], F32, tag="mm2")
                    for kc in range(8):
                        lhsT = w_[:, kc, mc * P:(mc + 1) * P]
                        nc.tensor.matmul(pt1[:], lhsT=lhsT, rhs=rhs1[:, kc, :],
                                         start=(kc == 0), stop=(kc == 7))
                        nc.tensor.matmul(pt2[:], lhsT=lhsT, rhs=rhs2[:, kc, :],
                                         start=(kc == 0), stop=(kc == 7))
                    nc.scalar.activation(out=o1_[:, mc, :], in_=pt1[:],
                                         func=Identity, bias=bias[:, mc:mc + 1])
                    nc.scalar.activation(out=o2_[:, mc, :], in_=pt2[:],
                                         func=Identity, bias=bias[:, mc:mc + 1])
                nc.sync.dma_start(out=out1_d[:], in_=o1_[:])
                nc.sync.dma_start(out=out2_d[:], in_=o2_[:])

            dual_mm("k", krT, ke_b, bias_sb["k"], out_d["khT"], out_d["khoT"])
            dual_mm("q", qrT, qx_b, bias_sb["q"], out_d["qhT"], out_d["qxhT"])

            w = load_w("v")
            vhT = act.tile([P, 8, TPC], BF16, tag="vhT")
            for mc, pt in mm_ws(w, ve_b):
                nc.scalar.activation(out=vhT[:, mc, :], in_=pt[:], func=Identity,
                                     bias=bias_sb["v"][:, mc:mc + 1])
            nc.sync.dma_start(out=out_d["vhT"][:], in_=vhT[:])

    nc.compile()
    return nc


# =====================================================================
# Phase-2 program (uniform across cores; causal masks are data)
# =====================================================================
def _build_phase2(repeat=1):
    nc = bacc.Bacc("TRN2", target_bir_lowering=False, debug=False)

    def din(name, shape, dt):
        return nc.dram_tensor(name, shape, dt, kind="ExternalInput").ap()

    def dout(name, shape, dt):
        return nc.dram_tensor(name, shape, dt, kind="ExternalOutput").ap()

    qhT_d = din("qhT", [P, 8, 256], BF16)
    qxhT_d = din("qxhT", [P, 8, 256], BF16)
    xn_d = din("xn", [2, P, D], F32)
    p0b_d = din("p0b", [1, H, 256], BF16)
    p0f_d = din("p0f", [1, H, 256], F32)
    mask_d = din("mask", [P, 8, P], BF16)
    khT_d = din("khT", [P, 8, N], BF16)
    khoT_d = din("khoT", [P, 8, N], BF16)
    vhx_d = din("vhx", [P, 8, H * 65], BF16)
    vsink_d = din("vsink", [1, H * 65], BF16)
    kxhT_d = din("kxhT", [P, 8, S], BF16)
    vxx_d = din("vxx", [P, 4, H * 65], BF16)
    qchT_d = din("qchT", [P, 8, P], BF16)
    statec_d = din("statec", [P, D], F32)
    ohcT_d = din("ohcT", [P, 8, P], F32)
    gb1_d = din("gb1", [P, 3 * D], BF16)
    gb2_d = din("gb2", [P, 3 * D], BF16)
    bo1T_d = din("bo1T", [P, 8], F32)
    w_d = {}
    for nm, kc, m in (("Wfs", 8, D), ("Wfc", 8, D), ("Wmv1", 8, 2 * D),
                      ("Wmv2", 16, D), ("Wolh", 8, D), ("Wg1", 8, 3 * D),
                      ("Wg2", 8, 3 * D), ("Wmh1", 8, 2 * D), ("Wmh2", 16, D)):
        w_d[nm] = din(nm, [P, kc, m], BF16)
    o2_d = dout("o2", [2, P, D], F32)
    c2_d = dout("c2", [P, D], F32)

    with tile.TileContext(nc) as tc:
        with tc.tile_pool(name="sb", bufs=1) as sb, \
             tc.tile_pool(name="wt", bufs=2) as wt, \
             tc.tile_pool(name="sm", bufs=2) as sm, \
             tc.tile_pool(name="pp", bufs=2) as ppool, \
             tc.tile_pool(name="ps", bufs=2, space="PSUM") as ps, \
             (tc.For_i(0, repeat, 1) if repeat > 1 else _nullctx()):

            ident_f = sb.tile([P, P], F32, tag="identf")
            make_identity(nc, ident_f)
            ident_b = sb.tile([P, P], BF16, tag="identb")
            make_identity(nc, ident_b)
            eps_t = sb.tile([P, 1], F32, tag="eps")
            nc.vector.memset(eps_t[:], EPS)

            def ld(d_ap, shape, dt, tag, name=None):
                t = sb.tile(shape, dt, tag=tag, name=name or tag)
                nc.sync.dma_start(out=t[:], in_=d_ap[:])
                return t

            qhT = ld(qhT_d, [P, 8, 256], BF16, "qhT")
            qxhT = ld(qxhT_d, [P, 8, 256], BF16, "qxhT")
            qchT = ld(qchT_d, [P, 8, P], BF16, "qchT")
            p0b = ld(p0b_d, [1, H, 256], BF16, "p0b")
            p0f = ld(p0f_d, [1, H, 256], F32, "p0f")
            mask = ld(mask_d, [P, 8, P], BF16, "mask")
            vhx = ld(vhx_d, [P, 8, H * 65], BF16, "vhx")
            vsink = ld(vsink_d, [1, H * 65], BF16, "vsink")
            vxx = ld(vxx_d, [P, 4, H * 65], BF16, "vxx")

            def attention(out_sb, out_qsl, kT_sb, q_sb, qsl, v_sb, nkc, W_,
                          tag, masked=(), mask_of=None, sink=None):
                """S^T attention for all 16 heads -> out_sb[:, hp, out_qsl]."""
                for h in range(H):
                    hp, base = h // 2, 64 * (h % 2)
                    p_t = ppool.tile([P, nkc, W_], BF16, tag=f"p_{tag}")
                    for kc in range(nkc):
                        pt = ps.tile([P, 256], F32, tag="s")
                        nc.tensor.matmul(
                            pt[:, :W_],
                            lhsT=kT_sb[base:base + 64, hp, kc * P:(kc + 1) * P],
                            rhs=q_sb[base:base + 64, hp, qsl],
                            start=True, stop=True, tile_position=(base, 0))
                        nc.scalar.activation(out=p_t[:, kc, :], in_=pt[:, :W_],
                                             func=Exp, scale=ISCALE)
                        if kc in masked:
                            nc.vector.tensor_tensor(
                                out=p_t[:, kc, :], in0=p_t[:, kc, :],
                                in1=mask[:, mask_of[kc], :W_], op=MULT)
                    av = ps.tile([65, 256], F32, tag="av")
                    for kc in range(nkc):
                        nc.tensor.matmul(av[:, :W_],
                                         lhsT=v_sb[:, kc, 65 * h:65 * h + 65],
                                         rhs=p_t[:, kc, :],
                                         start=(kc == 0),
                                         stop=(kc == nkc - 1 and sink is None))
                    r = sm.tile([1, 256], F32, tag="r")
                    if sink is not None:
                        nc.tensor.matmul(av[:, :W_],
                                         lhsT=vsink[0:1, 65 * h:65 * h + 65],
                                         rhs=p0b[0:1, h, sink],
                                         start=False, stop=True)
                        nc.vector.tensor_tensor(out=r[:, :W_], in0=av[64:65, :W_],
                                                in1=p0f[0:1, h, sink], op=ADD)
                        nc.vector.reciprocal(r[:, :W_], r[:, :W_])
                    else:
                        nc.vector.reciprocal(r[:, :W_], av[64:65, :W_])
                    R = sm.tile([64, 256], F32, tag="R")
                    nc.gpsimd.partition_broadcast(R[:, :W_], r[:, :W_])
                    nc.vector.tensor_tensor(
                        out=out_sb[base:base + 64, hp, out_qsl],
                        in0=av[0:64, :W_], in1=R[:, :W_], op=MULT)

            # ---------------- self attention (strided causal) ----------------
            khT = ld(khT_d, [P, 8, N], BF16, "khT")
            attnS = sb.tile([P, 8, 256], BF16, tag="attnS")
            for t in range(2):
                qsl = slice(t * P, t * P + P)
                mask_of = {kc: SELF_MASKED[t].index(kc) + 4 * t
                           for kc in SELF_MASKED[t]}
                attention(attnS, qsl, khT, qhT, qsl, vhx, SELF_NKC[t], P,
                          "self", masked=SELF_MASKED[t], mask_of=mask_of,
                          sink=qsl)

            # ---------------- och (state queries x token keys) ----------------
            khoT = ld(khoT_d, [P, 8, N], BF16, "khoT")
            ochT = sb.tile([P, 8, P], BF16, tag="ochT")
            attention(ochT, slice(0, P), khoT, qchT, slice(0, P), vhx, 8, P,
                      "och")

            # ---------------- cross (token queries x state keys) --------------
            kxhT = ld(kxhT_d, [P, 8, S], BF16, "kxhT")
            attnC = sb.tile([P, 8, 256], BF16, tag="attnC")
            attention(attnC, slice(0, 256), kxhT, qxhT, slice(0, 256), vxx, 4,
                      256, "cross")

            # ---------------- o1 ----------------
            bo1T = ld(bo1T_d, [P, 8], F32, "bo1T")
            xn_sb = sb.tile([P, 2, D], F32, tag="xn")
            for ti in range(2):
                nc.sync.dma_start(out=xn_sb[:, ti, :], in_=xn_d[ti])

            def stream_w(nm, kcn, mc, width, bufs=3):
                t = wt.tile([P, kcn, width], BF16, tag="wmc", bufs=bufs,
                            name=f"{nm}_{mc}")
                nc.sync.dma_start(out=t[:],
                                  in_=w_d[nm][:, :, mc * width:(mc + 1) * width])
                return t

            o1T = sb.tile([P, 8, 256], F32, tag="o1T")
            for mc in range(8):
                wfs = stream_w("Wfs", 8, mc, P)
                wfc = stream_w("Wfc", 8, mc, P)
                pt = ps.tile([P, 256], F32, tag="mm")
                for kc in range(8):
                    nc.tensor.matmul(pt[:], lhsT=wfs[:, kc, :],
                                     rhs=attnS[:, kc, :], start=(kc == 0),
                                     stop=False)
                for kc in range(8):
                    nc.tensor.matmul(pt[:], lhsT=wfc[:, kc, :],
                                     rhs=attnC[:, kc, :], start=False,
                                     stop=(kc == 7))
                nc.scalar.activation(out=o1T[:, mc, :], in_=pt[:], func=Identity,
                                     bias=bo1T[:, mc:mc + 1])
            o1 = sb.tile([P, 2, D], F32, tag="o1")
            for ti in range(2):
                for dc in range(8):
                    pt = ps.tile([P, P], F32, tag="tr")
                    nc.tensor.transpose(pt[:], o1T[:, dc, ti * P:(ti + 1) * P],
                                        ident_f[:])
                    nc.vector.tensor_tensor(
                        out=o1[:, ti, dc * P:(dc + 1) * P], in0=pt[:],
                        in1=xn_sb[:, ti, dc * P:(dc + 1) * P], op=ADD)

            # ---------------- MLP_v ----------------
            ln1 = sb.tile([P, 2, D], BF16, tag="lnv")
            for ti in range(2):
                _emit_ln(nc, sm, o1[:, ti, :], ln1[:, ti, :], eps_t)
            ln1T = sb.tile([P, 8, 256], BF16, tag="attnS")   # reuse slot
            for ti in range(2):
                for dc in range(8):
                    pt = ps.tile([P, P], BF16, tag="tr")
                    nc.tensor.transpose(pt[:], ln1[:, ti, dc * P:(dc + 1) * P],
                                        ident_b[:])
                    nc.scalar.activation(out=ln1T[:, dc, ti * P:(ti + 1) * P],
                                         in_=pt[:], func=CopyF)
            h1T = sb.tile([P, 16, 256], BF16, tag="khoT")    # reuse slot
            for mc in range(16):
                wmv = stream_w("Wmv1", 8, mc, P)
                pt = ps.tile([P, 256], F32, tag="mm")
                for kc in range(8):
                    nc.tensor.matmul(pt[:], lhsT=wmv[:, kc, :],
                                     rhs=ln1T[:, kc, :], start=(kc == 0),
                                     stop=(kc == 7))
                nc.scalar.activation(out=h1T[:, mc, :], in_=pt[:], func=Relu)
            o2T = sb.tile([P, 8, 256], F32, tag="o1T")       # reuse slot
            for mc in range(8):
                wmv = wt.tile([P, 16, P], BF16, tag="wmc", bufs=3,
                              name=f"Wmv2_{mc}")
                nc.sync.dma_start(out=wmv[:],
                                  in_=w_d["Wmv2"][:, :, mc * P:(mc + 1) * P])
                pt = ps.tile([P, 256], F32, tag="mm")
                for kc in range(16):
                    nc.tensor.matmul(pt[:], lhsT=wmv[:, kc, :],
                                     rhs=h1T[:, kc, :], start=(kc == 0),
                                     stop=(kc == 15))
                nc.scalar.activation(out=o2T[:, mc, :], in_=pt[:], func=CopyF)
            o2 = sb.tile([P, 2, D], F32, tag="xn")           # reuse slot
            for ti in range(2):
                for dc in range(8):
                    pt = ps.tile([P, P], F32, tag="tr")
                    nc.tensor.transpose(pt[:], o2T[:, dc, ti * P:(ti + 1) * P],
                                        ident_f[:])
                    nc.vector.tensor_tensor(
                        out=o2[:, ti, dc * P:(dc + 1) * P], in0=pt[:],
                        in1=o1[:, ti, dc * P:(dc + 1) * P], op=ADD)
                nc.sync.dma_start(out=o2_d[ti], in_=o2[:, ti, :])

            # ---------------- oh ----------------
            ohcT = ld(ohcT_d, [P, 8, P], F32, "ohcT")
            ohT = sb.tile([P, 8, P], BF16, tag="ohT")
            for mc in range(8):
                wol = stream_w("Wolh", 8, mc, P)
                pt = ps.tile([P, 256], F32, tag="mm")
                for kc in range(8):
                    nc.tensor.matmul(pt[:, :P], lhsT=wol[:, kc, :],
                                     rhs=ochT[:, kc, :], start=(kc == 0),
                                     stop=(kc == 7))
                nc.vector.tensor_tensor(out=ohT[:, mc, :], in0=pt[:, :P],
                                        in1=ohcT[:, mc, :], op=ADD)

            # ---------------- gates + MLP_h ----------------
            statec = sb.tile([P, D], F32, tag="attnC")       # reuse slot
            nc.sync.dma_start(out=statec[:], in_=statec_d[:])

            def gate_block(src_fm, wname, gbd, cprev, out_c):
                gb = ld(gbd, [P, 3 * D], BF16, "gb", name=wname + "_gb")
                gsb = sb.tile([P, 3 * D], F32, tag="khT")    # reuse slot
                for nt in range(6):
                    wg = wt.tile([P, 8, 512], BF16, tag="wnt", bufs=2,
                                 name=f"{wname}_{nt}")
                    nc.sync.dma_start(
                        out=wg[:], in_=w_d[wname][:, :, nt * 512:(nt + 1) * 512])
                    pt = ps.tile([P, 512], F32, tag="mm")
                    for kc in range(8):
                        nc.tensor.matmul(pt[:], lhsT=src_fm[:, kc, :],
                                         rhs=wg[:, kc, :],
                                         start=(kc == 0), stop=(kc == 7))
                    pre = sm.tile([P, 512], F32, tag="gpre", bufs=2)
                    nc.vector.tensor_tensor(out=pre[:], in0=pt[:],
                                            in1=gb[:, nt * 512:(nt + 1) * 512],
                                            op=ADD)
                    func = Tanh if nt >= 4 else Sigmoid
                    nc.scalar.activation(out=gsb[:, nt * 512:(nt + 1) * 512],
                                         in_=pre[:], func=func)
                iz = sm.tile([P, D], F32, tag="iz", bufs=1)
                nc.vector.tensor_tensor(out=iz[:], in0=gsb[:, 0:D],
                                        in1=gsb[:, 2 * D:3 * D], op=MULT)
                nc.vector.tensor_tensor(out=out_c[:], in0=gsb[:, D:2 * D],
                                        in1=cprev[:], op=MULT)
                nc.vector.tensor_tensor(out=out_c[:], in0=out_c[:], in1=iz[:],
                                        op=ADD)

            c1 = sb.tile([P, D], F32, tag="qhT")             # reuse slot
            gate_block(ohT, "Wg1", gb1_d, statec, c1)

            lnh = sb.tile([P, D], BF16, tag="qchT")          # reuse slot
            _emit_ln(nc, sm, c1[:], lnh[:], eps_t)
            lnhT = sb.tile([P, 8, P], BF16, tag="mask")      # reuse slot
            for dc in range(8):
                pt = ps.tile([P, P], BF16, tag="tr")
                nc.tensor.transpose(pt[:], lnh[:, dc * P:(dc + 1) * P], ident_b[:])
                nc.scalar.activation(out=lnhT[:, dc, :], in_=pt[:], func=CopyF)
            hh = sb.tile([P, 2 * D], BF16, tag="kxhT")       # reuse slot
            for nt in range(4):
                wg = wt.tile([P, 8, 512], BF16, tag="wnt", bufs=2,
                             name=f"Wmh1_{nt}")
                nc.sync.dma_start(
                    out=wg[:], in_=w_d["Wmh1"][:, :, nt * 512:(nt + 1) * 512])
                pt = ps.tile([P, 512], F32, tag="mm")
                for kc in range(8):
                    nc.tensor.matmul(pt[:], lhsT=lnhT[:, kc, :],
                                     rhs=wg[:, kc, :],
                                     start=(kc == 0), stop=(kc == 7))
                nc.scalar.activation(out=hh[:, nt * 512:(nt + 1) * 512],
                                     in_=pt[:], func=Relu)
            hhT = sb.tile([P, 16, P], BF16, tag="vhx")       # reuse slot
            for dc in range(16):
                pt = ps.tile([P, P], BF16, tag="tr")
                nc.tensor.transpose(pt[:], hh[:, dc * P:(dc + 1) * P], ident_b[:])
                nc.scalar.activation(out=hhT[:, dc, :], in_=pt[:], func=CopyF)
            mT = sb.tile([P, 8, P], BF16, tag="ohT")         # reuse slot
            for mc in range(8):
                wm2 = wt.tile([P, 16, P], BF16, tag="wmc", bufs=3,
                              name=f"Wmh2_{mc}")
                nc.sync.dma_start(out=wm2[:],
                                  in_=w_d["Wmh2"][:, :, mc * P:(mc + 1) * P])
                pt = ps.tile([P, 256], F32, tag="mm")
                for kc in range(16):
                    nc.tensor.matmul(pt[:, :P], lhsT=wm2[:, kc, :],
                                     rhs=hhT[:, kc, :], start=(kc == 0),
                                     stop=(kc == 15))
                nc.scalar.activation(out=mT[:, mc, :], in_=pt[:, :P], func=CopyF)

            c2 = sb.tile([P, D], F32, tag="qxhT")            # reuse slot
            gate_block(mT, "Wg2", gb2_d, c1, c2)
            nc.sync.dma_start(out=c2_d[:], in_=c2[:])

    nc.compile()
    return nc


# =====================================================================
# Host orchestration
# =====================================================================
def _host_pre(x, freqs, Pm):
    c_t = np.cos(freqs[WIN:WIN + N])
    s_t = np.sin(freqs[WIN:WIN + N])
    sgn = np.concatenate([-np.ones(DM // 2, np.float32),
                          np.ones(DM // 2, np.float32)])
    s_signed = s_t * sgn

    def rope_tabs(g):
        gs = np.concatenate([g[DM // 2:], g[:DM // 2]])
        return c_t * g, s_signed * gs

    cq, sq = rope_tabs(Pm['g_qnorm'])
    ck, sk = rope_tabs(Pm['g_knorm'])

    def ln0(t):
        m = t.mean(-1, keepdims=True)
        v = ((t - m) ** 2).mean(-1, keepdims=True)
        return (t - m) / np.sqrt(v + EPS)

    def heads(t):
        return t.reshape(*t.shape[:-1], H, Dh)

    def softmax(sc):
        m = sc.max(-1, keepdims=True)
        e = np.exp(sc - m)
        return e / e.sum(-1, keepdims=True)

    state = Pm['state']
    spi = Pm['spi']
    scn = ln0(state) * Pm['g_state_norm']
    kx = (scn + spi) @ Pm['W_Ks'] + Pm['b_Ks']
    vx = (scn + spi) @ Pm['W_Vs'] + Pm['b_Vs']
    kxh = kx @ Pm['Wk'] + Pm['bk']
    vxh = vx @ Pm['Wv'] + Pm['bv']
    qch_h = ((state + spi) @ Pm['W_Qeh'] + Pm['b_Qeh']) @ Pm['Wq'] + Pm['bq']
    qsh_h = ((scn + spi) @ Pm['W_Qsh'] + Pm['b_Qsh']) @ Pm['Wq'] + Pm['bq']
    s_sh = np.einsum('shd,khd->hsk', heads(qsh_h), heads(kxh)) / np.sqrt(Dh)
    osh_attn = np.einsum('hsk,khd->shd', softmax(s_sh),
                         heads(vxh)).reshape(S, DM)
    b_oh = Pm['bo'] @ Pm['W_lh'][:DM] + Pm['bo'] @ Pm['W_lh'][DM:] + Pm['b_lh']
    oh_const = osh_attn @ (Pm['Wo'] @ Pm['W_lh'][:DM]) + b_oh

    return dict(
        cq=cq, sq=sq, ck=ck, sk=sk, kxh=kxh, vxh=vxh, qch_h=qch_h,
        oh_const=oh_const,
        Wo_lv_c=Pm['Wo'] @ Pm['W_lv'][:DM],
        Wo_lv_s=Pm['Wo'] @ Pm['W_lv'][DM:],
        b_o1=(Pm['bo'] @ Pm['W_lv'][:DM] + Pm['bo'] @ Pm['W_lv'][DM:]
              + Pm['b_lv']),
        Wo_lh_c=Pm['Wo'] @ Pm['W_lh'][DM:],
        W_mv1=Pm['g_mv'][:, None] * Pm['W_mv1'],
        W_mh1=Pm['g_mh'][:, None] * Pm['W_mh1'],
    )


def _phase1_inmaps(x, hp, Pm):
    w1 = {"W_Ke": _wst(Pm['W_Ke']), "W_Ve": _wst(Pm['W_Ve']),
          "W_Qev": _wst(Pm['W_Qev']), "W_Qsv": _wst(Pm['W_Qsv']),
          "W_k": _wst(Pm['Wk']), "W_v": _wst(Pm['Wv']), "W_q": _wst(Pm['Wq'])}
    b1 = {"b_Ke": _bT(Pm['b_Ke']), "b_Ve": _bT(Pm['b_Ve']),
          "b_Qev": _bT(Pm['b_Qev']), "b_Qsv": _bT(Pm['b_Qsv']),
          "b_k": _bT(Pm['bk']), "b_v": _bT(Pm['bv']), "b_q": _bT(Pm['bq']),
          "gnT": _bT(Pm['g_norm'])}
    in1 = []
    for core in range(NCORES):
        b = core // 4
        t0 = (core % 4) * TPC
        m = {"x": np.ascontiguousarray(x[b, t0:t0 + TPC].reshape(2, P, D))}
        for nm in ("cq", "sq", "ck", "sk"):
            m[nm] = np.ascontiguousarray(
                hp[nm][t0:t0 + TPC].reshape(2, P, D)).astype(np.float32)
        m.update(w1)
        m.update(b1)
        in1.append(m)
    return in1


def _phase2_inmaps(r1, hp, Pm):
    def batch_cat(name, dtype):
        return [np.concatenate(
            [np.asarray(r1.results[4 * b + t][name], dtype) for t in range(4)],
            axis=2) for b in range(B)]

    khT_b = batch_cat("khT", np.float32)
    khoT_b = batch_cat("khoT", np.float32)
    qhT_b = batch_cat("qhT", np.float32)
    qxhT_b = batch_cat("qxhT", np.float32)
    vhT_b = batch_cat("vhT", np.float32)
    xnT_b = batch_cat("xnT", np.float32)

    def build_vext(v_tm, nkc):
        """v_tm [keys, DM] -> [128, nkc, H*65] with ones column."""
        ext = np.zeros((P, nkc, H * 65), np.float32)
        v3 = v_tm.reshape(nkc, P, H, Dh)
        for hh_ in range(H):
            ext[:, :, 65 * hh_:65 * hh_ + 64] = \
                v3[:, :, hh_, :].transpose(1, 0, 2)
            ext[:, :, 65 * hh_ + 64] = 1.0
        return ext

    vhx_np = [build_vext(_fm2full(vhT_b[b]).T, 8) for b in range(B)]
    vxx_np = build_vext(hp['vxh'], 4)
    vsink_np = np.zeros((1, H * 65), np.float32)
    for hh_ in range(H):
        vsink_np[0, 65 * hh_:65 * hh_ + 64] = Pm['bv'][64 * hh_:64 * hh_ + 64]

    p0_b = []
    for b in range(B):
        qh_tm = _fm2full(qhT_b[b]).T
        s0 = np.einsum('nhd,hd->nh', qh_tm.reshape(N, H, Dh),
                       Pm['bk'].reshape(H, Dh))
        p0 = (WIN - np.arange(N))[:, None] * np.exp(s0 * ISCALE)
        p0_b.append(p0.T)                                  # [H, N]

    # strided causal masks, identical structure for every core; values per j
    def build_mask(j):
        m = np.zeros((P, 8, P), np.float32)
        pp, ff = np.meshgrid(np.arange(P), np.arange(P), indexing='ij')
        idx = 0
        for t in range(2):
            for kc in SELF_MASKED[t]:
                m[:, idx, :] = ((kc * P + pp) <= (4 * (P * t + ff) + j))
                idx += 1
        return m

    gb1 = np.concatenate([Pm['b_i1'] - 1.0, Pm['b_f1'] + 1.0, Pm['b_z1']], 1)
    gb2 = np.concatenate([Pm['b_i2'] - 1.0, Pm['b_f2'] + 1.0, Pm['b_z2']], 1)
    w2 = {"Wfs": _wst(hp['Wo_lv_s']), "Wfc": _wst(hp['Wo_lv_c']),
          "Wmv1": _wst(hp['W_mv1']), "Wmv2": _wst(Pm['W_mv2']),
          "Wolh": _wst(hp['Wo_lh_c']),
          "Wg1": _wst(np.concatenate([Pm['W_i1'], Pm['W_f1'], Pm['W_z1']], 1)),
          "Wg2": _wst(np.concatenate([Pm['W_i2'], Pm['W_f2'], Pm['W_z2']], 1)),
          "Wmh1": _wst(hp['W_mh1']), "Wmh2": _wst(Pm['W_mh2'])}
    kxhT_fm = _full2fm(np.ascontiguousarray(hp['kxh'].T)).astype(BF)
    vxx_bf = vxx_np.astype(BF)

    in2 = []
    for core in range(NCORES):
        b, jj = core // 4, core % 4
        cols = jj + 4 * np.arange(256)                     # strided q rows
        xn_tm = _fm2full(xnT_b[b]).T
        m = {
            "qhT": np.ascontiguousarray(qhT_b[b][:, :, cols]).astype(BF),
            "qxhT": np.ascontiguousarray(qxhT_b[b][:, :, cols]).astype(BF),
            "xn": np.ascontiguousarray(xn_tm[cols].reshape(2, P, D)),
            "p0b": np.ascontiguousarray(p0_b[b][None, :, cols]).astype(BF),
            "p0f": np.ascontiguousarray(p0_b[b][None, :, cols]).astype(np.float32),
            "mask": build_mask(jj).astype(BF),
            "khT": khT_b[b].astype(BF),
            "khoT": khoT_b[b].astype(BF),
            "vhx": vhx_np[b].astype(BF),
            "vsink": vsink_np.astype(BF),
            "kxhT": kxhT_fm,
            "vxx": vxx_bf,
            "qchT": _full2fm(np.ascontiguousarray(
                hp['qch_h'].T[:, jj * P:(jj + 1) * P])).astype(BF),
            "statec": np.ascontiguousarray(
                Pm['state'][jj * P:(jj + 1) * P]).astype(np.float32),
            "ohcT": _full2fm(np.ascontiguousarray(
                hp['oh_const'][jj * P:(jj + 1) * P].T)).astype(np.float32),
            "gb1": np.ascontiguousarray(gb1[jj * P:(jj + 1) * P]).astype(BF),
            "gb2": np.ascontiguousarray(gb2[jj * P:(jj + 1) * P]).astype(BF),
            "bo1T": _bT(hp['b_o1']),
        }
        m.update(w2)
        in2.append(m)
    return in2


def _assemble(r2):
    out = np.zeros((B, N + S, D), np.float32)
    for core in range(NCORES):
        b, jj = core // 4, core % 4
        rows = jj + 4 * np.arange(256)
        o2 = np.asarray(r2.results[core]["o2"], np.float32).reshape(256, D)
        out[b, rows] = o2
        out[b, N + jj * P:N + (jj + 1) * P] = \
            np.asarray(r2.results[core]["c2"], np.float32)
    return out


def kernel(x, freqs, params):
    x = np.asarray(x, np.float32)
    freqs = np.asarray(freqs, np.float32)
    Pm = {k: np.asarray(v, np.float32) for k, v in params.items()}
    hp = _host_pre(x, freqs, Pm)
    if "p1" not in _PROGS:
        _PROGS["p1"] = _build_phase1()
    if "p2" not in _PROGS:
        _PROGS["p2"] = _build_phase2()
    in1 = _phase1_inmaps(x, hp, Pm)
    r1 = run_bass_kernel_spmd(_PROGS["p1"], in1, core_ids=list(range(NCORES)))
    in2 = _phase2_inmaps(r1, hp, Pm)
    r2 = run_bass_kernel_spmd(_PROGS["p2"], in2, core_ids=list(range(NCORES)))
    return _assemble(r2)
